# revision 8
# baseline (speedup 1.0000x reference)
"""Multi-head attention (B=4, N=2048, C=1024, H=16, D=64) on 8 trn2 cores.

Sharding: core c handles batch b = c//2 and head-group g = c%2 (8 heads,
512 channels). No collectives: the row-parallel output projection partials
are summed on host (2 cores per batch), with bp + bv@Wp folded in on host
(softmax rows sum to 1, so the v-bias contributes a constant per channel).

Device layout is transposed end-to-end (tokens on the free axis):
  xT [C, N] -> Q^T/K^T pair tiles [128, N] (rows 0:64 head 2p, 64:128 head
  2p+1) -> S^T tiles [keys, queries] via row-packed K=64 matmul pairs ->
  exp on ACT (no max subtraction; scores are O(1) by construction) ->
  PV with V_aug [vA | ones64 | vB]: psum rows 0:64 = O_u / rowsum and
  64:128 = rowsum / O_u -> DVE reciprocal * mul -> O^T -> Y^T = Wp_g^T O^T.

All matmul operands are float32r (1 cycle/row at free >= 256, ~2^-12
relative rounding), accumulation fp32 in PSUM.
"""

import os
import sys

sys.path.insert(0, "/opt/trn_rl_repo")

import numpy as np

B, N, C, H = 4, 2048, 1024, 16
D = C // H
SCALE = D**-0.5
NCORES = 8
FC = 512  # channels per core
NP = 4  # head pairs per core
KT8 = C // 128  # 8 contraction tiles
NCQ = N // 512  # 4 n-chunks of 512

# matmul operand dtype: "bf16" (2x PE throughput, ~5e-3 rel err) or
# "f32r" (~1e-4 rel err, PE-bound)
MM_DT = os.environ.get("KERNEL_MM_DT", "bf16")

_nc = None


def _cap(ap_slice, block_step, nblocks, width):
    """2-free-dim AP: nblocks blocks of `width` cols, block stride block_step,
    starting at ap_slice (a [128, width] slice)."""
    import concourse.bass as bass

    lst = [list(p) for p in ap_slice.ap]
    assert len(lst) == 2 and lst[1][0] == 1, lst
    return bass.AP(ap_slice.tensor, ap_slice.offset, [lst[0], [block_step, nblocks], [1, width]])


def _build():
    import concourse.bacc as bacc
    import concourse.mybir as mybir
    import concourse.tile as tile

    F32 = mybir.dt.float32
    MDT = mybir.dt.bfloat16 if MM_DT == "bf16" else mybir.dt.float32r
    AF = mybir.ActivationFunctionType

    nc = bacc.Bacc("TRN2", target_bir_lowering=False, debug=False, num_devices=NCORES)

    xT_d = nc.dram_tensor("xT", (C, N), MDT, kind="ExternalInput").ap()
    wq_d = nc.dram_tensor("wq", (C, FC), MDT, kind="ExternalInput").ap()
    wk_d = nc.dram_tensor("wk", (C, FC), MDT, kind="ExternalInput").ap()
    wv_d = nc.dram_tensor("wv", (C, FC), MDT, kind="ExternalInput").ap()
    wp_d = nc.dram_tensor("wp", (FC, C), MDT, kind="ExternalInput").ap()
    bq_d = nc.dram_tensor("bq", (128, NP), F32, kind="ExternalInput").ap()
    bk_d = nc.dram_tensor("bk", (128, NP), F32, kind="ExternalInput").ap()
    on_d = nc.dram_tensor("ones", (128, 256), MDT, kind="ExternalInput").ap()
    yT_d = nc.dram_tensor("yT", (C, N), F32, kind="ExternalOutput").ap()

    with tile.TileContext(nc) as tc:
        with tc.tile_pool(name="persist", bufs=1) as pp:
            QT = [pp.tile([128, N], MDT, name=f"qt{p}") for p in range(NP)]
            KT = [pp.tile([128, N], MDT, name=f"kt{p}") for p in range(NP)]
            VA = [pp.tile([128, 192 * NP], MDT, name=f"va{t}") for t in range(N // 128)]
            bq_t = pp.tile([128, NP], F32, name="bq_t")
            bk_t = pp.tile([128, NP], F32, name="bk_t")
            on_t = pp.tile([128, 256], MDT, name="on_t")
            nc.sync.dma_start(out=bq_t[:], in_=bq_d)
            nc.sync.dma_start(out=bk_t[:], in_=bk_d)
            nc.sync.dma_start(out=on_t[:], in_=on_d)

            # ---------------- phase A: Q^T, K^T, V_aug ----------------
            with (
                tc.tile_pool(name="pa_w", bufs=1) as paw,
                tc.tile_pool(name="pa_xt", bufs=10) as pax,
                tc.tile_pool(name="pa_ps", bufs=4, space="PSUM") as paps,
                tc.tile_pool(name="pa_psv", bufs=2, space="PSUM") as papsv,
            ):
                wq_t = [paw.tile([128, FC], MDT, name=f"wq{k}") for k in range(KT8)]
                wk_t = [paw.tile([128, FC], MDT, name=f"wk{k}") for k in range(KT8)]
                wv_t = [paw.tile([128, FC], MDT, name=f"wv{k}") for k in range(KT8)]
                for k in range(KT8):
                    nc.sync.dma_start(out=wq_t[k][:], in_=wq_d[128 * k : 128 * (k + 1), :])
                    nc.sync.dma_start(out=wk_t[k][:], in_=wk_d[128 * k : 128 * (k + 1), :])
                    nc.sync.dma_start(out=wv_t[k][:], in_=wv_d[128 * k : 128 * (k + 1), :])

                for ncq in range(NCQ):
                    cs = slice(512 * ncq, 512 * (ncq + 1))
                    xts = []
                    for k in range(KT8):
                        xt = pax.tile([128, 512], MDT, tag="xt", name=f"xt_{ncq}_{k}")
                        nc.sync.dma_start(out=xt[:], in_=xT_d[128 * k : 128 * (k + 1), cs])
                        xts.append(xt)
                    for p in range(NP):
                        ps_ = slice(128 * p, 128 * (p + 1))
                        pq = paps.tile([128, 512], F32, tag="psqk", name=f"pq_{ncq}_{p}")
                        for k in range(KT8):
                            nc.tensor.matmul(pq[:], wq_t[k][:, ps_], xts[k][:], start=(k == 0), stop=(k == KT8 - 1))
                        nc.scalar.activation(QT[p][:, cs], pq[:], AF.Identity, bias=bq_t[:, p : p + 1])
                        pk = paps.tile([128, 512], F32, tag="psqk", name=f"pk_{ncq}_{p}")
                        for k in range(KT8):
                            nc.tensor.matmul(pk[:], wk_t[k][:, ps_], xts[k][:], start=(k == 0), stop=(k == KT8 - 1))
                        nc.scalar.activation(KT[p][:, cs], pk[:], AF.Identity, bias=bk_t[:, p : p + 1])
                    for t in range(4):
                        nt = 4 * ncq + t
                        ts_ = slice(128 * t, 128 * (t + 1))
                        pv = papsv.tile([128, 512], F32, tag="psv", name=f"pv_{nt}")
                        for k in range(KT8):
                            nc.tensor.matmul(pv[:], xts[k][:, ts_], wv_t[k][:], start=(k == 0), stop=(k == KT8 - 1))
                        va = VA[nt]
                        # scatter: vA cols 128p:+64 -> 192p; vB 128p+64:+64 -> 192p+128
                        nc.vector.tensor_copy(
                            _cap(va[:, 0:64], 192, NP, 64), _cap(pv[:, 0:64], 128, NP, 64)
                        )
                        nc.vector.tensor_copy(
                            _cap(va[:, 128:192], 192, NP, 64), _cap(pv[:, 64:128], 128, NP, 64)
                        )
                        nc.vector.tensor_copy(_cap(va[:, 64:128], 192, NP, 64), on_t[:])

            # ---------------- phase B: attention ----------------
            with tc.tile_pool(name="pot", bufs=1) as pot:
                OT = [pot.tile([128, N], MDT, name=f"ot{p}") for p in range(NP)]
                with (
                    tc.tile_pool(name="pb_e", bufs=4) as pbe,
                    tc.tile_pool(name="pb_rc", bufs=4) as prc,
                    tc.tile_pool(name="pb_psa", bufs=1, space="PSUM") as psa,
                    tc.tile_pool(name="pb_psb", bufs=1, space="PSUM") as psb,
                    tc.tile_pool(name="pb_pso", bufs=4, space="PSUM") as pso,
                ):
                    for p in range(NP):
                        for half in range(2):
                            n0 = 1024 * half
                            ots = [
                                pso.tile([128, 512], F32, tag="o", name=f"o_{p}_{half}_{j}")
                                for j in range(4)
                            ]
                            # HAM warm-up: the PE clock gate needs ~3.4us of
                            # continuous matmul work to release 2.4GHz; the
                            # ACT-gated steady state never provides it, but
                            # once warm it holds. Burn a burst at strip entry
                            # (results discarded - first real PV has start=True).
                            nkick = 20 if (p == 0 and half == 0) else 4
                            for w in range(nkick):
                                nc.tensor.matmul(
                                    ots[w % 4], KT[p][:, 0:128], QT[p][:, n0 : n0 + 512],
                                    start=True, stop=True, skip_group_check=True,
                                )
                            for mt in range(N // 128):
                                ms = slice(128 * mt, 128 * (mt + 1))
                                sa = psa.tile([128, 1024], F32, tag="sa", name=f"sa_{p}_{half}_{mt}")
                                sb_ = psb.tile([128, 1024], F32, tag="sb", name=f"sb_{p}_{half}_{mt}")
                                for j in range(2):
                                    qs = slice(n0 + 512 * j, n0 + 512 * (j + 1))
                                    es = slice(512 * j, 512 * (j + 1))
                                    nc.tensor.matmul(
                                        sa[:, es], KT[p][0:64, ms], QT[p][0:64, qs],
                                        start=True, stop=True,
                                    )
                                    nc.tensor.matmul(
                                        sb_[:, es], KT[p][64:128, ms], QT[p][64:128, qs],
                                        start=True, stop=True,
                                    )
                                ea = pbe.tile([128, 1024], MDT, tag="e", name=f"ea_{p}_{half}_{mt}")
                                eb = pbe.tile([128, 1024], MDT, tag="e", name=f"eb_{p}_{half}_{mt}")
                                nc.scalar.activation(ea[:], sa[:], AF.Exp, scale=SCALE)
                                nc.scalar.activation(eb[:], sb_[:], AF.Exp, scale=SCALE)
                                va = VA[mt]
                                first, last = mt == 0, mt == N // 128 - 1
                                for j in range(2):
                                    es = slice(512 * j, 512 * (j + 1))
                                    nc.tensor.matmul(
                                        ots[j], va[:, 192 * p : 192 * p + 128], ea[:, es],
                                        start=first, stop=last, skip_group_check=True,
                                    )
                                    nc.tensor.matmul(
                                        ots[2 + j], va[:, 192 * p + 64 : 192 * p + 192], eb[:, es],
                                        start=first, stop=last, skip_group_check=True,
                                    )
                            # normalize: j<2 head A (O rows 0:64, rowsum 64:128),
                            # j>=2 head B (rowsum 0:64, O 64:128)
                            for j in range(4):
                                o = ots[j]
                                col = slice(n0 + 512 * (j % 2), n0 + 512 * (j % 2) + 512)
                                if j < 2:
                                    osl, rsl, dst = o[0:64, :], o[64:128, :], OT[p][0:64, col]
                                else:
                                    osl, rsl, dst = o[64:128, :], o[0:64, :], OT[p][64:128, col]
                                rc = prc.tile([64, 512], F32, tag="rc", name=f"rc_{p}_{half}_{j}")
                                nc.vector.reciprocal(rc[:], rsl)
                                nc.vector.tensor_mul(dst, osl, rc[:])

                # ---------------- phase C: output projection ----------------
                with (
                    tc.tile_pool(name="pc_wp", bufs=1) as pwp,
                    tc.tile_pool(name="pc_y", bufs=4) as pyb,
                    tc.tile_pool(name="pc_ps", bufs=4, space="PSUM") as psy,
                ):
                    wp_t = [pwp.tile([128, C], MDT, name=f"wp{f}") for f in range(NP)]
                    for f in range(NP):
                        nc.sync.dma_start(out=wp_t[f][:], in_=wp_d[128 * f : 128 * (f + 1), :])
                    for c in range(C // 128):
                        for ncq in range(NCQ):
                            cs = slice(512 * ncq, 512 * (ncq + 1))
                            py = psy.tile([128, 512], F32, tag="y", name=f"py_{c}_{ncq}")
                            for f in range(NP):
                                nc.tensor.matmul(
                                    py[:], wp_t[f][:, 128 * c : 128 * (c + 1)], OT[f][:, cs],
                                    start=(f == 0), stop=(f == NP - 1),
                                )
                            yb = pyb.tile([128, 512], F32, tag="yb", name=f"yb_{c}_{ncq}")
                            nc.scalar.activation(yb[:], py[:], AF.Copy)
                            nc.sync.dma_start(out=yT_d[128 * c : 128 * (c + 1), cs], in_=yb[:])

    nc.compile()
    return nc


def _get_nc():
    global _nc
    if _nc is None:
        try:
            import jax

            jax.config.update(
                "jax_compilation_cache_dir", os.path.expanduser("~/.cache/jax_bass")
            )
            jax.config.update("jax_persistent_cache_min_compile_time_secs", 0.0)
            jax.config.update("jax_persistent_cache_min_entry_size_bytes", 0)
        except Exception:
            pass
        _nc = _build()
    return _nc


def make_in_maps(inputs):
    if MM_DT == "bf16":
        import ml_dtypes

        mdt = ml_dtypes.bfloat16
    else:
        mdt = np.float32
    x = np.asarray(inputs["x"], np.float32)
    Wq = np.asarray(inputs["Wq"], np.float32)
    Wk = np.asarray(inputs["Wk"], np.float32)
    Wv = np.asarray(inputs["Wv"], np.float32)
    Wp = np.asarray(inputs["Wp"], np.float32)
    bq = np.asarray(inputs["bq"], np.float32)
    bk = np.asarray(inputs["bk"], np.float32)
    ones = np.ones((128, 256), mdt)
    in_maps = []
    for core in range(NCORES):
        b, g = core // 2, core % 2
        sl = slice(FC * g, FC * (g + 1))
        in_maps.append(
            {
                "xT": np.ascontiguousarray(x[b].T.astype(mdt)),
                "wq": np.ascontiguousarray(Wq[:, sl].astype(mdt)),
                "wk": np.ascontiguousarray(Wk[:, sl].astype(mdt)),
                "wv": np.ascontiguousarray(Wv[:, sl].astype(mdt)),
                "wp": np.ascontiguousarray(Wp[sl, :].astype(mdt)),
                "bq": np.ascontiguousarray(bq[sl].reshape(NP, 128).T),
                "bk": np.ascontiguousarray(bk[sl].reshape(NP, 128).T),
                "ones": ones,
            }
        )
    return in_maps


def assemble(results, inputs):
    Wp = np.asarray(inputs["Wp"], np.float32)
    bv = np.asarray(inputs["bv"], np.float32)
    bp = np.asarray(inputs["bp"], np.float32)
    fb = (bp.astype(np.float64) + bv.astype(np.float64) @ Wp.astype(np.float64)).astype(
        np.float32
    )
    out = np.empty((B, N, C), np.float32)
    for b in range(B):
        yt = results[2 * b]["yT"] + results[2 * b + 1]["yT"]
        out[b] = yt.T + fb
    return out


def run_on_device(inputs, trace=False, tmpdir=None):
    from concourse.bass_utils import run_bass_kernel_spmd

    nc = _get_nc()
    res = run_bass_kernel_spmd(
        nc, make_in_maps(inputs), list(range(NCORES)), trace=trace, tmpdir=tmpdir
    )
    return assemble(res.results, inputs), res


def kernel(**inputs):
    out, _ = run_on_device(inputs)
    return out
